# revision 54
# baseline (speedup 1.0000x reference)
"""Trainium2 Bass kernel for nn_DGMMLoss (retrieval_knn) — fused single launch.

Reference computation:
  1. x_ul = lam*x + (1-lam)*x[perm]; pseudo-labels = mode of 11-NN labels
  2. concat; per-class means; gaussian-mixture loss rows
  3. knn regularizer: mode of 3-NN (self-excluded) labels, MSE
  loss = loss_gm + 0.01 * loss_knn

Strategy (8 NeuronCores, ONE SPMD launch, data-parallel over query rows):
  - Core c owns 1024 rows of xc=[x; x_ul]: blocks 0-3 = x rows, 4-7 = x_ul
    rows. Scores S[q, r] = q.r - ||r||^2/2 over all 8192 refs via fp8(e4m3)
    DoubleRow matmuls (256-deep contraction at 0.5 cyc/row; host-validated
    rel err ~4.4e-3 vs the 2e-2 gate) with -||r||^2/2 folded in as a bf16
    hi/lo rank-2 matmul into the same psum accumulation. S stays f32.
  - Phase A (11-NN pseudo-labels) on the x_ul blocks' first score half
    (r < 4096): DVE max8 + match_replace + max8 give the 11th-largest
    threshold; masks are fp8, PE-transposed (stride-2 psum), counted
    against a Pool-built fp8 one-hot via fp8 DoubleRow matmuls; mode =
    first argmax (smallest class on ties, matching torch.mode). The A max8
    also seeds the phase-B threshold (second half rescanned only).
  - ONE AllReduce carries everything across cores: the bf16 message packs
    per-core partial mu sums [C,512], partial class counts [C,1] (integer,
    exact in bf16), and y_ul in per-core slots (zero elsewhere, so add ==
    gather). Label-independent phase-B work for ALL blocks (thresholds,
    Pool mask compares, transposes, first-half counts vs known labels) is
    emitted before the collective's consumers so in-order engine queues
    never head-of-line block on it.
  - Phase B (k=4 incl self): second-half counts after the collective,
    self-label exclusion, mode; GM rows pi = exp(q.mu - aa/2)*emu row-
    normalized (fp8 DoubleRow q.mu; emu broadcast via gpsimd), per-row
    sum((pi - onehot)^2).
Host does only O(N*D) packing and the final scalar means.
"""

from contextlib import ExitStack

import numpy as np
import ml_dtypes

import concourse.bacc as bacc
import concourse.tile as tile
import concourse.mybir as mybir
from concourse.bass_utils import run_bass_kernel_spmd
from concourse.masks import make_identity

P = 128
NCORES = 8
CLASSES = 100
N = 4096
D = 512
NUM = 2 * N
F32 = mybir.dt.float32
BF16 = mybir.dt.bfloat16
FP8 = mybir.dt.float8e4
BF16_NP = ml_dtypes.bfloat16
FP8_NP = ml_dtypes.float8_e4m3
ALU = mybir.AluOpType
AX = mybir.AxisListType
PM = mybir.MatmulPerfMode

QPC = NUM // NCORES          # queries per core (1024)
QB = QPC // P                # query blocks per core (8; 0-3 = x, 4-7 = x_ul)
RT = NUM // P                # ref chunks of 128 (64)
RCH = NUM // 512             # ref chunks of 512 (16)
HT = N // P                  # half-row chunks of 128 (32)
C = CLASSES
ARW = D + 4 * NCORES         # AllReduce message width (544)


def build_program(n_cores=NCORES):
    nc = bacc.Bacc(
        "TRN2", target_bir_lowering=False, debug=False, num_devices=n_cores
    )
    # fp8 refs for DoubleRow: pair t covers dims [t*256,(t+1)*256);
    # element (p, j, r) = xc[r, t*256 + j*128 + p]
    xc8_ap = [
        nc.dram_tensor(f"xc8_{t}", [P, 2, NUM], FP8, kind="ExternalInput").ap()
        for t in range(2)
    ]
    q8_ap = [
        nc.dram_tensor(f"q8_{t}", [P, 2, QPC], FP8, kind="ExternalInput").ap()
        for t in range(2)
    ]
    bb_ap = nc.dram_tensor("bbhl", [2, NUM], BF16, kind="ExternalInput").ap()
    # labels of the known half, packed [P, 32]: col i = y[i*128:(i+1)*128]
    yl_ap = nc.dram_tensor("ylab", [P, HT], F32, kind="ExternalInput").ap()
    io_ap = nc.dram_tensor("iotaf", [P, C], F32, kind="ExternalInput").ap()
    # col 0..7: -aa/2 per query block; col 8..11: labels of x blocks
    qa_ap = nc.dram_tensor("qaux", [P, QB + 4], F32, kind="ExternalInput").ap()
    # this core's query rows in [r_p, d] layout (mu partials)
    xr_ap = nc.dram_tensor("xrows", [P, QB, D], BF16, kind="ExternalInput").ap()
    # 0/1 selector with ones in this core's 4 y_ul slot columns
    zs_ap = nc.dram_tensor("zsel", [P, 4 * NCORES], F32, kind="ExternalInput").ap()

    yul_ap = nc.dram_tensor("yulx", [P, 4 * NCORES], F32, kind="ExternalOutput").ap()
    yng_ap = nc.dram_tensor("yng", [QB, P, 1], F32, kind="ExternalOutput").ap()
    lg_ap = nc.dram_tensor("lgm", [QB, P, 1], F32, kind="ExternalOutput").ap()

    ar_in = nc.dram_tensor("ar_in", [P, ARW], BF16)
    ar_out = nc.dram_tensor("ar_out", [P, ARW], BF16, addr_space="Shared")
    rg = [list(range(n_cores))]

    with tile.TileContext(nc) as tc, ExitStack() as ctx:
        consts = ctx.enter_context(tc.tile_pool(name="consts", bufs=1))
        spool = ctx.enter_context(tc.tile_pool(name="spool", bufs=2))
        mpool = ctx.enter_context(tc.tile_pool(name="mpool", bufs=2))
        mtpool = ctx.enter_context(tc.tile_pool(name="mtpool", bufs=1))
        small = ctx.enter_context(tc.tile_pool(name="small", bufs=1))
        psS_p = ctx.enter_context(tc.tile_pool(name="psS", bufs=2, space="PSUM"))
        psT_p = ctx.enter_context(tc.tile_pool(name="psT", bufs=2, space="PSUM"))
        psC_p = ctx.enter_context(tc.tile_pool(name="psC", bufs=2, space="PSUM"))

        # ---- input DMAs, critical-path first: q8, ref groups, bb ----
        GROUP = 2048
        NG = NUM // GROUP
        q8 = []
        for t in range(2):
            tl = consts.tile([P, 2, QPC], FP8, name=f"q8_{t}", tag=f"q8_{t}")
            nc.sync.dma_start(tl[:, :, QPC // 2:QPC],
                              q8_ap[t][:, :, QPC // 2:QPC])
            q8.append(tl)
        xc8 = [[None] * NG for _ in range(2)]
        # bb hi/lo pairs at partition offsets 0 and 32 (matmul base-partition
        # constraint) so the tile costs only 8KB of free space per partition
        bbt8 = consts.tile([34, NUM // 2], BF16, name="bbt8", tag="bbt8")
        for g in range(NG):
            for t in range(2):
                tl = consts.tile([P, 2, GROUP], FP8, name=f"xc8_{t}_{g}",
                                 tag=f"xc8_{t}_{g}")
                nc.sync.dma_start(
                    tl[:], xc8_ap[t][:, :, g * GROUP:(g + 1) * GROUP]
                )
                xc8[t][g] = tl
            jg, off = g // 2, (g % 2) * GROUP
            nc.sync.dma_start(
                bbt8[32 * jg:32 * jg + 2, off:off + GROUP],
                bb_ap[:, g * GROUP:(g + 1) * GROUP])

        for t in range(2):
            nc.sync.dma_start(q8[t][:, :, 0:QPC // 2],
                              q8_ap[t][:, :, 0:QPC // 2])
        identb = consts.tile([P, P], BF16, name="identb", tag="identb")
        make_identity(nc, identb)
        ident8 = consts.tile([P, P], FP8, name="ident8", tag="ident8")
        make_identity(nc, ident8)
        ones2 = consts.tile([34, P], BF16, name="ones2", tag="ones2")
        nc.vector.memset(ones2[:], 1.0)

        # tiny touches absorb DMA-queue waits into dedicated copies
        tchV = consts.tile([1, 1], F32, name="tchV", tag="tchV")
        tchA = consts.tile([1, 1], F32, name="tchA", tag="tchA")
        tchP = consts.tile([1, 1], F32, name="tchP", tag="tchP")

        def dve_touch(t):
            nc.vector.tensor_copy(tchV[:], t[0:1, 0:1])

        def act_touch(t):
            nc.scalar.copy(tchA[:], t[0:1, 0:1])

        def pool_touch(t):
            nc.gpsimd.tensor_copy(tchP[:], t[0:1, 0:1])

        ylabt = consts.tile([P, RT], F32, name="ylabt", tag="ylabt")
        nc.sync.dma_start(ylabt[:, 0:HT], yl_ap[:])
        iot = consts.tile([P, C], F32, name="iot", tag="iot")
        nc.sync.dma_start(iot[:], io_ap[:])
        qauxt = consts.tile([P, QB + 4], F32, name="qauxt", tag="qauxt")
        nc.sync.dma_start(qauxt[:], qa_ap[:])
        xrows = consts.tile([P, QB, D], BF16, name="xrows", tag="xrows")
        nc.sync.dma_start(xrows[:], xr_ap[:])
        zselt = consts.tile([P, 4 * NCORES], F32, name="zselt", tag="zselt")
        nc.sync.dma_start(zselt[:], zs_ap[:])
        pool_touch(iot)
        pool_touch(ylabt)
        pool_touch(qauxt)
        dve_touch(iot)
        dve_touch(zselt)
        act_touch(qauxt)

        # fp8 one-hot of ref labels (0/1 exact in e4m3); known chunks now
        yoht = consts.tile([P, RT, C], FP8, name="yoht", tag="yoht")
        for i in range(HT):
            nc.gpsimd.tensor_scalar(
                out=yoht[:, i, :], in0=iot[:], scalar1=ylabt[:, i:i + 1],
                scalar2=None, op0=ALU.is_equal,
            )
        # bf16 one-hot of this core's own 1024 rows (mu partials); x half now
        ohown = consts.tile([P, QB, C], BF16, name="ohown", tag="ohown")
        for b in range(4):
            nc.gpsimd.tensor_scalar(
                out=ohown[:, b, :], in0=iot[:],
                scalar1=qauxt[:, QB + b:QB + b + 1],
                scalar2=None, op0=ALU.is_equal,
            )

        S_tiles = {}
        m8h1 = {}
        t4col = {}
        mTk2 = {}        # kept second-half maskT, all 8 blocks
        cfirst = {}      # staged first-half counts, all 8 blocks
        yul_tiles = {}

        def emit_scores(b):
            S = spool.tile([P, NUM], F32, name="S", tag="S")
            for jj in range(RCH // 2):
                ps = psS_p.tile([P, 1024], F32, name="psS", tag="psS")
                for h in range(2):
                    j = 2 * jj + h
                    g, go = (j * 512) // GROUP, (j * 512) % GROUP
                    for t in range(2):
                        nc.tensor.matmul(
                            ps[:, h * 512:(h + 1) * 512],
                            q8[t][:, :, b * P:(b + 1) * P],
                            xc8[t][g][:, :, go:go + 512],
                            start=(t == 0), stop=False,
                            perf_mode=PM.DoubleRow,
                            skip_group_check=True,
                        )
                    jg, off = (j * 512) // (NUM // 2), (j * 512) % (NUM // 2)
                    nc.tensor.matmul(
                        ps[:, h * 512:(h + 1) * 512],
                        ones2[32 * jg:32 * jg + 2, :],
                        bbt8[32 * jg:32 * jg + 2, off:off + 512],
                        start=False, stop=True,
                        skip_group_check=True,
                    )
                nc.scalar.copy(S[:, jj * 1024:(jj + 1) * 1024], ps[:])
            S_tiles[b] = S
            return S

        def emit_B_threshold(b):
            """t4 = 4th largest of the full row; x_ul blocks reuse the
            phase-A first-half max8."""
            assert b not in t4col
            S = S_tiles[b]
            m16 = small.tile([P, 16], F32, name="m16", tag="m16", bufs=1)
            if b >= 4:
                nc.vector.tensor_copy(m16[:, 0:8], m8h1.pop(b)[:])
            else:
                nc.vector.max(out=m16[:, 0:8], in_=S[:, 0:N])
            nc.vector.max(out=m16[:, 8:16], in_=S[:, N:NUM])
            mm8 = small.tile([P, 8], F32, name="mm8", tag="mm8", bufs=2)
            nc.vector.max(out=mm8[:], in_=m16[:])
            t4col[b] = mm8

        def emit_mask(b, mt, col, r_hi, engines):
            S = S_tiles[b]
            mask = mpool.tile([P, NUM], FP8, name="maskB", tag="maskB")
            half = r_hi // 2
            for h in range(2):
                engines[h].tensor_scalar(
                    out=mask[:, h * half:(h + 1) * half],
                    in0=S[:, h * half:(h + 1) * half],
                    scalar1=mt[:, col:col + 1], scalar2=None, op0=ALU.is_ge,
                )
            return mask

        def emit_transposes(mask, c_lo, c_hi, mT, mt_base):
            GT = 8
            for i0 in range(c_lo, c_hi, GT):
                pst = psT_p.tile([P, GT, P, 2], FP8, name="psT", tag="psT")
                for u in range(GT):
                    nc.tensor.transpose(
                        pst[:, u, :, 0],
                        mask[:, (i0 + u) * P:(i0 + u + 1) * P],
                        ident8[:],
                    )
                o = mt_base + i0 - c_lo
                nc.scalar.copy(mT[:, o:o + GT, :], pst[:, :, :, 0])

        def emit_counts(mT, npair, oh_base, psc):
            for j in range(npair):
                nc.tensor.matmul(
                    psc[:],
                    mT[:, 2 * j:2 * j + 2, :],
                    yoht[:, oh_base + 2 * j:oh_base + 2 * j + 2, :],
                    start=(j == 0), stop=(j == npair - 1),
                    perf_mode=PM.DoubleRow,
                )

        def emit_mode(counts_ap, tagp, out_ap, eng, ym_tag="ym"):
            """First-argmax (smallest class on ties). eng=nc.vector uses the
            fused scalar_tensor_tensor; the Pool variant stays entirely on
            gpsimd so output-only tails run parallel to DVE work."""
            maxc = small.tile([P, 1], F32, name="mx", tag="mx", bufs=2)
            nc.vector.reduce_max(maxc[:], counts_ap, axis=AX.X)
            lt01 = small.tile([P, C], F32, name="lt", tag="lt", bufs=1)
            eng.tensor_scalar(
                out=lt01[:], in0=counts_ap, scalar1=maxc[:], scalar2=None,
                op0=ALU.is_lt,
            )
            cand = small.tile([P, C], F32, name="cd", tag="cd", bufs=1)
            if eng is nc.vector:
                nc.vector.scalar_tensor_tensor(
                    out=cand[:], in0=lt01[:], scalar=1e9, in1=iot[:],
                    op0=ALU.mult, op1=ALU.add,
                )
            else:
                eng.tensor_scalar(
                    out=cand[:], in0=lt01[:], scalar1=1e9, scalar2=None,
                    op0=ALU.mult,
                )
                eng.tensor_add(cand[:], cand[:], iot[:])
            ym = small.tile([P, 1], F32, name=f"ym{tagp}", tag=ym_tag,
                            bufs=3 if ym_tag == "ym" else 1)
            nc.vector.tensor_reduce(ym[:], cand[:], axis=AX.X, op=ALU.min)
            if out_ap is not None:
                nc.sync.dma_start(out_ap, ym[:])
            return ym

        def emit_B_prep(b, cmp):
            """Label-independent phase-B work: mask, transposes, first-half
            counts vs known labels (staged to SBUF)."""
            if b not in t4col:
                emit_B_threshold(b)
            maskB = emit_mask(b, t4col[b], 3, NUM, cmp)
            mTt = mtpool.tile([P, HT, P], FP8, name="mTt", tag="mTt")
            emit_transposes(maskB, 0, HT, mTt, 0)
            psc = psC_p.tile([P, 512], F32, name="psC", tag="psC")
            emit_counts(mTt, HT // 2, 0, psc[:, 0:C])
            cf = small.tile([P, C], F32, name=f"cf{b}", tag=f"cf{b}")
            nc.vector.tensor_copy(cf[:], psc[:, 0:C])
            cfirst[b] = cf
            mk = consts.tile([P, HT, P], FP8, name=f"mTk{b}", tag=f"mTk{b}")
            emit_transposes(maskB, HT, RT, mk, 0)
            mTk2[b] = mk

        # ---------------- stage 1: x_ul blocks, phase-A critical path ----
        Ssc = consts.tile([P, N], F32, name="Ssc", tag="Ssc")
        agp = small.tile([P, 4], F32, name="agp", tag="agp")

        def emit_A_head(u, m2):
            # 11-NN mask over refs [0, N); 11th largest = m2 col 2
            mask = emit_mask(u, m2, 2, N, (nc.vector, nc.vector))
            mTt = mtpool.tile([P, HT, P], FP8, name="mTt", tag="mTt")
            emit_transposes(mask, 0, HT, mTt, 0)
            psc = psC_p.tile([P, 512], F32, name="psC", tag="psC")
            emit_counts(mTt, HT // 2, 0, psc[:, 0:C])
            return psc

        def emit_A_finish(u, psc):
            cnt = small.tile([P, C], F32, name="cntA", tag="cntA", bufs=1)
            nc.vector.tensor_copy(cnt[:], psc[:, 0:C])
            ym = emit_mode(cnt[:], f"A{u}", None, nc.vector,
                           ym_tag=f"yma{u}")
            yul_tiles[u] = ym
            nc.vector.tensor_copy(agp[:, u - 4:u - 3], ym[:])
            nc.vector.tensor_scalar(
                out=ohown[:, u, :], in0=iot[:], scalar1=ym[:],
                scalar2=None, op0=ALU.is_equal,
            )

        def emit_collective():
            psmu = psC_p.tile([P, 512], F32, name="psmu", tag="psC")
            for b in range(QB):
                nc.tensor.matmul(
                    psmu[0:C, 0:D], ohown[:, b, :], xrows[:, b, :],
                    start=(b == 0), stop=(b == QB - 1),
                )
            musum = small.tile([P, ARW], BF16, name="musum", tag="musum")
            nc.vector.memset(musum[:], 0.0)
            nc.vector.tensor_copy(musum[0:C, 0:D], psmu[0:C, 0:D])
            for g in range(n_cores):
                nc.vector.tensor_mul(
                    musum[:, D + g * 4:D + (g + 1) * 4],
                    agp[:], zselt[:, g * 4:(g + 1) * 4],
                )
            nc.gpsimd.dma_start(ar_in.ap(), musum[:])
            nc.gpsimd.collective_compute(
                kind="AllReduce", op=ALU.add, replica_groups=rg,
                ins=[ar_in.ap()], outs=[ar_out.ap()],
            )

        for u in range(4, 8):
            emit_scores(u)
            S = S_tiles[u]
            m1 = small.tile([P, 8], F32, name="m1", tag="m1", bufs=2)
            nc.vector.max(out=m1[:], in_=S[:, 0:N])
            m8h1[u] = m1
            nc.vector.match_replace(
                out=Ssc[:], in_to_replace=m1[:], in_values=S[:, 0:N],
                imm_value=-1e30,
            )
            m2 = small.tile([P, 8], F32, name="m2", tag="m2", bufs=2)
            nc.vector.max(out=m2[:], in_=Ssc[:])
            psc_u = emit_A_head(u, m2)
            if u == 7:
                emit_A_finish(u, psc_u)
                emit_collective()
                emit_B_prep(u, (nc.vector, nc.gpsimd))
            else:
                emit_B_prep(u, (nc.gpsimd, nc.gpsimd))
                emit_A_finish(u, psc_u)

        # ---------------- stages 2+3 interleaved ------------------------
        # bf16 message [P, 544]: rows 0:100 cols 0:512 = partial mu sums;
        # cols 512:544 = y_ul in this core's slots (zero elsewhere -> add
        # behaves as a gather). Class counts are recomputed from the global
        # one-hot after the collective instead of riding in the message.
        # The collective issue is sandwiched between x-block score sweeps
        # so neither PE nor the DVE compares stall on it.
        for b in range(4):
            emit_scores(b)
            emit_B_prep(b, (nc.vector, nc.gpsimd))

        # ---------------- stage 4: ingest collective + finish ------------
        arfull = small.tile([P, ARW], BF16, name="arfull", tag="arfull")
        nc.sync.dma_start(arfull[:], ar_out.ap())
        dve_touch(arfull)
        pool_touch(arfull)
        nc.vector.tensor_copy(ylabt[:, HT:RT], arfull[:, D:ARW])
        nc.sync.dma_start(yul_ap[:], ylabt[:, HT:RT])
        for i in range(HT, RT):
            nc.gpsimd.tensor_scalar(
                out=yoht[:, i, :], in0=iot[:], scalar1=ylabt[:, i:i + 1],
                scalar2=None, op0=ALU.is_equal,
            )
        onesq = consts.tile([P, 1], FP8, name="onesq", tag="onesq")
        nc.vector.memset(onesq[:], 1.0)
        pscc = psC_p.tile([P, 512], F32, name="psC", tag="psC")
        for i in range(RT):
            nc.tensor.matmul(
                pscc[0:C, 0:1], yoht[:, i, :], onesq[:],
                start=(i == 0), stop=(i == RT - 1),
            )
        ccnt = small.tile([C, 1], F32, name="ccnt", tag="ccnt")
        nc.vector.tensor_copy(ccnt[:], pscc[0:C, 0:1])
        cnt1 = small.tile([C, 1], F32, name="cnt1", tag="cnt1")
        nc.vector.tensor_scalar(
            out=cnt1[:], in0=ccnt[:], scalar1=1.0, scalar2=None,
            op0=ALU.max,
        )
        rcnt = small.tile([C, 1], F32, name="rcnt", tag="rcnt")
        nc.vector.reciprocal(rcnt[:], cnt1[:])
        mub = small.tile([C, D], BF16, name="mub", tag="mub")
        nc.vector.tensor_scalar(
            out=mub[:], in0=arfull[0:C, 0:D], scalar1=rcnt[:], scalar2=None,
            op0=ALU.mult,
        )
        # muT8 [P, 2, 2, C] fp8 via PE transposes; (t, j) matches q8 pairs
        mu8 = small.tile([C, D], FP8, name="mu8", tag="mu8")
        nc.vector.tensor_copy(mu8[:], mub[:])
        muT8 = small.tile([P, 2, 2, C], FP8, name="muT8", tag="muT8")
        pstm = psT_p.tile([P, 8, P, 2], FP8, name="psT", tag="psT")
        for dch in range(4):
            nc.tensor.transpose(
                pstm[:, dch, 0:C, 0], mu8[:, dch * P:(dch + 1) * P],
                ident8[0:C, 0:C],
            )
        nc.scalar.copy(muT8[:], pstm[:, 0:4, 0:C, 0])
        # emu = exp(-|mu|^2/2) * (counts > 0), broadcast to [P, C] f32
        mu2 = small.tile([C, 1], F32, name="mu2", tag="mu2")
        musq = small.tile([C, D], FP8, name="musq", tag="musq")
        nc.scalar.activation(
            musq[:], mub[:], mybir.ActivationFunctionType.Square,
            accum_out=mu2[:],
        )
        emuc = small.tile([C, 1], F32, name="emuc", tag="emuc")
        nc.scalar.activation(
            emuc[:], mu2[:], mybir.ActivationFunctionType.Exp,
            bias=0.0, scale=-0.5,
        )
        nz = small.tile([C, 1], F32, name="nz", tag="nz")
        nc.vector.tensor_scalar(
            out=nz[:], in0=ccnt[:], scalar1=0.0, scalar2=None,
            op0=ALU.is_gt,
        )
        emuz = small.tile([C, 1], F32, name="emuz", tag="emuz")
        nc.vector.tensor_mul(emuz[:], emuc[:], nz[:])
        emurow = small.tile([1, C], F32, name="emurow", tag="emurow")
        nc.sync.dma_start(emurow[:], emuz[:])
        emuB = consts.tile([P, C], F32, name="emuB", tag="emuB")
        nc.gpsimd.partition_broadcast(emuB[:], emurow[:])

        def emit_gm(b, yh):
            psgt = psC_p.tile([P, 512], F32, name="psC", tag="psC")
            psg = psgt[:, 0:C]
            for t in range(2):
                nc.tensor.matmul(
                    psg,
                    q8[t][:, :, b * P:(b + 1) * P],
                    muT8[:, t, :, :],
                    start=(t == 0), stop=(t == 1),
                    perf_mode=PM.DoubleRow,
                )
            eg = small.tile([P, C], F32, name="eg", tag="eg", bufs=1)
            nc.scalar.activation(
                eg[:], psg, mybir.ActivationFunctionType.Exp,
                bias=qauxt[:, b:b + 1], scale=1.0,
            )
            piu = small.tile([P, C], F32, name="piu", tag="piu", bufs=1)
            nc.vector.tensor_mul(piu[:], eg[:], emuB[:])
            srow = small.tile([P, 1], F32, name="sr", tag="sr", bufs=2)
            nc.vector.reduce_sum(srow[:], piu[:], axis=AX.X)
            nc.vector.tensor_scalar_add(srow[:], srow[:], 1e-15)
            rec = small.tile([P, 1], F32, name="rc", tag="rc", bufs=2)
            nc.vector.reciprocal(rec[:], srow[:])
            pin = small.tile([P, C], F32, name="pi", tag="pi", bufs=1)
            nc.vector.tensor_scalar(
                out=pin[:], in0=piu[:], scalar1=rec[:], scalar2=None,
                op0=ALU.mult,
            )
            diff = small.tile([P, C], F32, name="df", tag="df", bufs=1)
            nc.vector.tensor_sub(diff[:], pin[:], yh[:])
            sq = small.tile([P, C], F32, name="sq", tag="sq", bufs=1)
            nc.vector.tensor_mul(sq[:], diff[:], diff[:])
            lg = small.tile([P, 1], F32, name="lg", tag="lg", bufs=2)
            nc.vector.reduce_sum(lg[:], sq[:], axis=AX.X)
            nc.sync.dma_start(lg_ap[b], lg[:])

        def emit_B_tail(b):
            psc = psC_p.tile([P, 512], F32, name="psC", tag="psC")
            emit_counts(mTk2.pop(b), HT // 2, HT, psc[:, 0:C])
            yh = small.tile([P, C], F32, name=f"yh{b}", tag="yh", bufs=3)
            if b >= 4:
                lab = yul_tiles[b][:]
            else:
                lab = qauxt[:, QB + b:QB + b + 1]
            nc.vector.tensor_scalar(
                out=yh[:], in0=iot[:], scalar1=lab, scalar2=None,
                op0=ALU.is_equal,
            )
            counts = small.tile([P, C], F32, name="cntB", tag="cntB", bufs=1)
            nc.vector.tensor_sub(counts[:], psc[:, 0:C], yh[:])
            nc.vector.tensor_add(counts[:], counts[:], cfirst[b][:])
            emit_mode(counts[:], f"B{b}", yng_ap[b], nc.vector)
            emit_gm(b, yh)

        for b in [4, 5, 6, 7, 0, 1, 2, 3]:
            emit_B_tail(b)
    nc.compile()
    return nc


# ---------------- host-side packing ----------------

def pack_pairs_fp8(m):
    """[R, D] fp32 -> two fp8 tiles [P, 2, R]: tile t (p, j, r) =
    m[r, t*256 + j*128 + p]."""
    mt = np.ascontiguousarray(m.T.astype(FP8_NP))  # [D, R]
    out = []
    for t in range(2):
        a = mt[t * 256:t * 256 + P]
        b = mt[t * 256 + P:t * 256 + 2 * P]
        out.append(np.ascontiguousarray(np.stack([a, b], axis=1)))
    return out


def pack_bbhl(bb):
    t = (-0.5 * bb).astype(np.float32)
    hi = t.astype(BF16_NP)
    lo = (t - hi.astype(np.float32)).astype(BF16_NP)
    return np.ascontiguousarray(np.stack([hi, lo]))


def pack_cols(v):
    nb = v.shape[0] // P
    return np.ascontiguousarray(v.reshape(nb, P).T.astype(np.float32))


_PROGRAMS = {}
LAST_EXEC_NS = None


def _get_program():
    if "fused" not in _PROGRAMS:
        _PROGRAMS["fused"] = build_program()
    return _PROGRAMS["fused"]


def _qsel(c):
    return np.concatenate([
        np.arange(c * 512, (c + 1) * 512),
        np.arange(N + c * 512, N + (c + 1) * 512),
    ])


def kernel(x, y, lam, perm):
    import os

    global LAST_EXEC_NS
    x = np.asarray(x, dtype=np.float32)
    y = np.asarray(y, dtype=np.float32)
    lam = np.float32(np.asarray(lam))
    perm = np.asarray(perm, dtype=np.int32)
    x_ul = (x * lam + x[perm] * (np.float32(1.0) - lam)).astype(np.float32)
    xc = np.concatenate([x, x_ul], axis=0)
    aa = (xc.astype(np.float64) ** 2).sum(1).astype(np.float32)

    nc = _get_program()
    xc8 = pack_pairs_fp8(xc)
    bb_in = pack_bbhl(aa)
    ylab_in = pack_cols(y)
    iota_in = np.ascontiguousarray(
        np.broadcast_to(np.arange(C, dtype=np.float32), (P, C))
    )
    in_maps = []
    for c in range(NCORES):
        qsel = _qsel(c)
        xq = xc[qsel]
        q8 = pack_pairs_fp8(xq)
        qaux = np.concatenate(
            [pack_cols((-0.5 * aa[qsel]).astype(np.float32)),
             pack_cols(y[c * 512:(c + 1) * 512])], axis=1
        ).astype(np.float32)
        xrows = np.ascontiguousarray(
            xq.reshape(QB, P, D).transpose(1, 0, 2).astype(BF16_NP))
        zsel = np.zeros((P, 4 * NCORES), dtype=np.float32)
        zsel[:, c * 4:(c + 1) * 4] = 1.0
        in_maps.append({
            "xc8_0": xc8[0], "xc8_1": xc8[1],
            "q8_0": q8[0], "q8_1": q8[1],
            "bbhl": bb_in, "ylab": ylab_in, "iotaf": iota_in,
            "qaux": np.ascontiguousarray(qaux),
            "xrows": np.ascontiguousarray(xrows),
            "zsel": zsel,
        })

    kwargs = {}
    if os.environ.get("KERNEL_TRACE"):
        kwargs = dict(trace=True, trace_cores=[0])
    res = run_bass_kernel_spmd(nc, in_maps, core_ids=list(range(NCORES)),
                               **kwargs)
    if res.exec_time_ns:
        LAST_EXEC_NS = res.exec_time_ns

    # yulx col j = labels of x_ul rows [j*128, (j+1)*128)
    y_ul = res.results[0]["yulx"].T.reshape(N).astype(np.float32)
    yc = np.concatenate([y, y_ul])

    y_ng = np.zeros(NUM, dtype=np.float32)
    lgm = np.zeros(NUM, dtype=np.float64)
    for c in range(NCORES):
        qsel = _qsel(c)
        y_ng[qsel] = res.results[c]["yng"].reshape(QPC)
        lgm[qsel] = res.results[c]["lgm"].reshape(QPC)

    loss_gm = lgm.mean()
    loss_knn = ((y_ng - yc) ** 2).mean(dtype=np.float64)
    return np.float32(loss_gm + 0.01 * loss_knn)
